# revision 42
# baseline (speedup 1.0000x reference)
"""FBank preprocessor (kaldi-style log-mel) as a Bass/Trainium2 kernel.

Pipeline per 1-sec waveform (48 kHz):
  frame (98 x 1200, hop 480) -> remove DC -> preemphasis 0.97 -> hann
  -> zero-pad 2048 -> |rfft|^2 -> mel (128 banks) -> log -> pad 98->128
  -> (x - MEAN) / (2*STD)

Everything up to the power spectrum is linear in the frame samples, so
DC-removal/preemphasis/hann/rDFT are folded into one dense (1280 x 2048)
cos/sin matrix on the host, quantized to fp8e4m3 (tolerance is 2e-2;
fp8 end-to-end sims at ~4e-3).  On-device per 4-waveform block:
  - one strided DMA load of raw frames per waveform (t on partitions)
  - PE transposes to samples-on-partitions, DVE casts to fp8
  - fp8 DoubleRow matmuls (contract 256/instr) against the folded DFT
    matrix; each weight tile is shared by a group of 3 blocks so the
    LDWEIGHTS stream stays behind the matmuls
  - next group's transposes are interleaved between DFT weight
    segments to keep the PE stream dense (HAM throttle avoidance)
  - ACT Square PSUM->SBUF bf16, DVE add -> power spectrum
  - mel matmul (mel bank stationary), clamp/log/scale epilogue,
    PE transpose back, DMA out

Data parallel over 8 NeuronCores: 64 waveforms each.
"""

import functools

import numpy as np

import concourse.bass as bass
import concourse.bacc as bacc
import concourse.tile as tile
from concourse import mybir
from concourse import bass_utils

F32 = mybir.dt.float32
F32R = mybir.dt.float32r
BF16 = mybir.dt.bfloat16
FP8 = mybir.dt.float8e4

SR = 48000
WIN = 1200
HOP = 480
PADWIN = 2048
NMEL = 128
TFRAMES = 128
NFRAMES = 98
PREEMPH = 0.97
MEAN = -4.2677393
STD = 4.5689974
EPS = 1.1920928955078125e-07

NCORES = 8
B = 512
BPC = B // NCORES          # 64 waveforms per core
NCHUNK = 10                # contract chunks of 128 samples (1280 >= 1200)
NCP = 5                    # DoubleRow chunk pairs (contract 256 each)
KBINS = 1024               # rfft bins 0..1023 (bin 1024 has zero mel weight)
NW = 4                     # waveforms per block
NF = NW * NFRAMES          # 392 frames per block
NBLOCKS = BPC // NW        # 16
GRP = 2                    # blocks per weight-reuse group (3 dft psum bufs
                           # minus 2 in flight leaves 1 spare bank, so a
                           # half-tile's matmuls never wait on the previous
                           # half's squares)
MVW = 400                  # padded moving free stride (mult of 16 >= NF)

DSCALE = 4.0               # fp8 weight scale, compensated in mel bank
EPS_S = float(EPS * np.exp(-MEAN))
OUT_SCALE = float(1.0 / (2.0 * STD))
PADV = float((0.0 - MEAN) / (2.0 * STD))


def _mel_banks_f64():
    # torchaudio.compliance.kaldi.get_mel_banks (low 20 Hz, high nyquist)
    fft_bin_width = SR / PADWIN
    mel = lambda f: 1127.0 * np.log(1.0 + f / 700.0)
    mel_low, mel_high = mel(20.0), mel(SR / 2.0)
    delta = (mel_high - mel_low) / (NMEL + 1)
    left = mel_low + np.arange(NMEL)[:, None] * delta
    center = left + delta
    right = center + delta
    m = mel(fft_bin_width * np.arange(KBINS))[None, :]
    up = (m - left) / (center - left)
    down = (right - m) / (right - center)
    return np.maximum(0.0, np.minimum(up, down))  # (128, 1024)


def _build_host_constants():
    import ml_dtypes
    # T = diag(hann) @ P_preemph @ (I - ones/WIN), all (WIN x WIN), f64
    n = np.arange(WIN)
    hann = 0.5 - 0.5 * np.cos(2.0 * np.pi * n / (WIN - 1))
    T = np.eye(WIN) - np.ones((WIN, WIN)) / WIN
    P = np.eye(WIN)
    P[np.arange(1, WIN), np.arange(WIN - 1)] -= PREEMPH
    P[0, 0] -= PREEMPH            # kaldi replicate pad: first sample pairs itself
    T = P @ T
    T = hann[:, None] * T

    k = np.arange(KBINS)
    ang = 2.0 * np.pi * np.outer(n, k) / PADWIN      # (1200, 1024)
    d_re = T.T @ np.cos(ang)                          # (1200, 1024)
    d_im = T.T @ (-np.sin(ang))

    D = np.zeros((NCHUNK * 128, 2 * KBINS), np.float64)
    D[:WIN, :KBINS] = d_re
    D[:WIN, KBINS:] = d_im
    D8 = (D * DSCALE).astype(ml_dtypes.float8_e4m3)   # (1280, 2048)
    # DoubleRow weights: [partition n%128, chunk-pair, k-tile j, 2048 cols]
    dftm = np.ascontiguousarray(
        D8.reshape(NCP, 2, 128, 2 * KBINS).transpose(2, 0, 1, 3))

    fbs = _mel_banks_f64() * (np.exp(-MEAN) / (DSCALE * DSCALE))
    fbd = np.zeros((128, 8, 128), np.float64)
    for kk in range(8):
        fbd[:, kk, :] = fbs[:, kk * 128:(kk + 1) * 128].T
    return dftm, np.ascontiguousarray(
        fbd.astype(np.float32), dtype=ml_dtypes.bfloat16)


@functools.lru_cache(maxsize=1)
def _build_nc():
    nc = bacc.Bacc("TRN2", target_bir_lowering=False, debug=False,
                   num_devices=NCORES)

    WAVE = nc.dram_tensor("wave", [BPC, SR], F32R, kind="ExternalInput")
    DFT = nc.dram_tensor("dftm", [128, NCP, 2, 2 * KBINS], FP8,
                         kind="ExternalInput")
    FBD = nc.dram_tensor("fbd", [128, 8, 128], BF16, kind="ExternalInput")
    IDT = nc.dram_tensor("ident", [128, 128], F32R, kind="ExternalInput")
    IDTB = nc.dram_tensor("identb", [128, 128], BF16, kind="ExternalInput")
    OUT = nc.dram_tensor("out", [BPC, TFRAMES, NMEL], F32,
                         kind="ExternalOutput")

    def wave_ap(offset, dims):
        return bass.AP(tensor=WAVE, offset=offset, ap=list(dims))

    def out_ap(offset, dims):
        return bass.AP(tensor=OUT, offset=offset, ap=list(dims))

    DR = mybir.MatmulPerfMode.DoubleRow

    groups = [list(range(g, min(g + GRP, NBLOCKS)))
              for g in range(0, NBLOCKS, GRP)]

    with tile.TileContext(nc) as tc:
        with tc.tile_pool(name="const", bufs=1) as constp, \
             tc.tile_pool(name="ld", bufs=16) as ldp, \
             tc.tile_pool(name="mv", bufs=2 * GRP * NCP) as mvp, \
             tc.tile_pool(name="sq", bufs=12) as sqp, \
             tc.tile_pool(name="pw", bufs=9 * GRP) as pwp, \
             tc.tile_pool(name="epi", bufs=4) as epp, \
             tc.tile_pool(name="oc", bufs=4) as ocp, \
             tc.tile_pool(name="tr_ps", bufs=2, space="PSUM") as trps, \
             tc.tile_pool(name="dft_ps", bufs=3, space="PSUM") as dftps, \
             tc.tile_pool(name="mel_ps", bufs=2, space="PSUM") as melps:

            ident = constp.tile([128, 128], F32R)
            nc.sync.dma_start(out=ident[:], in_=IDT.ap())
            identb = constp.tile([128, 128], BF16)
            nc.sync.dma_start(out=identb[:], in_=IDTB.ap())
            ld_pending = {}
            ld_state = {"next": 0}

            def issue_ld():
                bg = ld_state["next"]
                if bg >= NBLOCKS:
                    return
                ld_state["next"] = bg + 1
                tiles = []
                for wb in range(NW):
                    lt = ldp.tile([NFRAMES, NCHUNK * 128], F32R, tag="ld",
                                  name=f"ld_{bg}_{wb}")
                    nc.sync.dma_start(
                        out=lt[:],
                        in_=wave_ap((bg * NW + wb) * SR,
                                    [[HOP, NFRAMES], [1, NCHUNK * 128]]),
                    )
                    tiles.append(lt)
                ld_pending[bg] = tiles

            for _ in range(GRP):
                issue_ld()

            dftm = constp.tile([128, NCP, 2, 2 * KBINS], FP8)
            for cp in range(NCP):
                nc.sync.dma_start(out=dftm[:, cp, :, :],
                                  in_=DFT.ap()[:, cp, :, :])
            fbd = constp.tile([128, 8, 128], BF16)
            nc.scalar.dma_start(out=fbd[:], in_=FBD.ap())
            PADG = 8  # waveforms per pad DMA
            padt = constp.tile([TFRAMES - NFRAMES, PADG, NMEL], F32)
            nc.vector.memset(padt[:], PADV)

            mv_tiles = {}

            def make_tr_thunks(bg, warm=False):
                # 10 chunk-thunks: 4 PE transposes + 1 fp8 cast each;
                # group 0's casts split DVE/ACT to halve the serial
                # warmup chain (ACT queue is empty before the squares)
                state = {}

                def thunk(cp, j, c):
                    if c == 0:
                        state["ld"] = ld_pending.pop(bg)
                        issue_ld()
                    ld = state["ld"]
                    if j == 0:
                        mv_tiles[(bg, cp)] = mvp.tile(
                            [128, 2, MVW], FP8, tag="mv",
                            name=f"mv_{bg}_{cp}")
                    mt = mv_tiles[(bg, cp)]
                    trp = trps.tile([128, NF], F32R, tag="trp",
                                    name=f"trp_{bg}_{c}")
                    for wb in range(NW):
                        nc.tensor.transpose(
                            trp[:, wb * NFRAMES:(wb + 1) * NFRAMES],
                            ld[wb][:, c * 128:(c + 1) * 128],
                            ident[:NFRAMES, :NFRAMES],
                        )
                    if warm and c % 2 == 1:
                        nc.scalar.copy(mt[:, j, :NF], trp[:])
                    else:
                        nc.vector.tensor_copy(mt[:, j, :NF], trp[:])

                return [functools.partial(thunk, cp, j, 2 * cp + j)
                        for cp in range(NCP) for j in range(2)]

            # prime: transposes for group 0
            pending = []
            for bg in groups[0]:
                pending.extend(make_tr_thunks(bg, warm=True))
            for t in pending:
                t()

            for gi, grp in enumerate(groups):
                # queue next group's transposes, interleaved into DFT below
                pending = []
                if gi + 1 < len(groups):
                    for bg in groups[gi + 1]:
                        pending.extend(make_tr_thunks(bg))
                per_seg = (len(pending) + 15) // 16 if pending else 0

                # ---- folded DFT, fp8 DoubleRow, weights shared over grp ----
                pw = {}
                for kk in range(8):
                    sq = {}
                    for half in range(2):
                        base = half * KBINS + kk * 128
                        pss = [dftps.tile([128, NF], F32, tag="dftps",
                                          name=f"dft_{gi}_{kk}_{half}_{i}")
                               for i in range(len(grp))]
                        for cp in range(NCP):
                            w = dftm[:, cp, :, base:base + 128]
                            for i, bg in enumerate(grp):
                                nc.tensor.matmul(
                                    pss[i][:], w, mv_tiles[(bg, cp)][:, :, :NF],
                                    start=(cp == 0), stop=(cp == NCP - 1),
                                    perf_mode=DR,
                                )
                        for i in range(len(grp)):
                            st = sqp.tile([128, NF], BF16, tag="sq",
                                          name=f"sq_{gi}_{kk}_{half}_{i}")
                            nc.scalar.square(st[:], pss[i][:])
                            sq[(i, half)] = st
                        # keep the PE stream dense: next group's transposes
                        for t in pending[:per_seg]:
                            t()
                        pending = pending[per_seg:]
                    for i in range(len(grp)):
                        pt = pwp.tile([128, NF], BF16, tag="pw",
                                      name=f"pw_{gi}_{kk}_{i}")
                        nc.vector.tensor_add(pt[:], sq[(i, 0)][:],
                                             sq[(i, 1)][:])
                        pw[(i, kk)] = pt
                for t in pending:
                    t()
                for bg in grp:
                    for cp in range(NCP):
                        del mv_tiles[(bg, cp)]

                # ---- mel (bank stationary), log-scale, transpose, out ----
                for i, bg in enumerate(grp):
                    mel = melps.tile([128, NF], F32, tag="mel",
                                     name=f"mel_{gi}_{i}")
                    for kk in range(8):
                        nc.tensor.matmul(mel[:], fbd[:, kk, :], pw[(i, kk)][:],
                                         start=(kk == 0), stop=(kk == 7))
                    ot = epp.tile([128, NF], F32R, tag="ot",
                                  name=f"ot_{gi}_{i}")
                    nc.vector.tensor_scalar_max(ot[:], mel[:], EPS_S)
                    nc.scalar.activation(ot[:], ot[:],
                                         mybir.ActivationFunctionType.Ln)
                    ob = epp.tile([128, NF], BF16, tag="ob",
                                  name=f"ob_{gi}_{i}")
                    nc.vector.tensor_scalar_mul(ob[:], ot[:], OUT_SCALE)
                    otr = trps.tile([NFRAMES, NW * 128], BF16, tag="otr",
                                    bufs=1, name=f"otr_{gi}_{i}")
                    for wb in range(NW):
                        nc.tensor.transpose(
                            otr[:, wb * 128:(wb + 1) * 128],
                            ob[:, wb * NFRAMES:(wb + 1) * NFRAMES],
                            identb[:],
                        )
                    oc = ocp.tile([NFRAMES, NW, NMEL], F32, tag="oc",
                                  name=f"oc_{gi}_{i}")
                    nc.vector.tensor_copy(oc[:], otr[:].rearrange(
                        "p (w m) -> p w m", w=NW))
                    nc.sync.dma_start(
                        out=out_ap(bg * NW * TFRAMES * NMEL,
                                   [[NMEL, NFRAMES],
                                    [TFRAMES * NMEL, NW],
                                    [1, NMEL]]),
                        in_=oc[:],
                    )

            # constant pad rows (frames 98..127) for every waveform
            for g0 in range(0, BPC, PADG):
                nc.scalar.dma_start(
                    out=out_ap(g0 * TFRAMES * NMEL + NFRAMES * NMEL,
                               [[NMEL, TFRAMES - NFRAMES],
                                [TFRAMES * NMEL, PADG],
                                [1, NMEL]]),
                    in_=padt[:],
                )

    nc.compile()
    return nc


@functools.lru_cache(maxsize=1)
def _host_constants():
    return _build_host_constants()


def kernel(waveform):
    waveform = np.ascontiguousarray(np.asarray(waveform, dtype=np.float32))
    assert waveform.shape == (B, SR), waveform.shape
    nc = _build_nc()
    dftm, fbd = _host_constants()
    shards = waveform.reshape(NCORES, BPC, SR)
    ident = np.eye(128, dtype=np.float32)
    import ml_dtypes
    identb = np.eye(128, dtype=ml_dtypes.bfloat16)
    in_maps = [
        {"wave": np.ascontiguousarray(shards[c]), "dftm": dftm, "fbd": fbd,
         "ident": ident, "identb": identb}
        for c in range(NCORES)
    ]
    res = bass_utils.run_bass_kernel_spmd(
        nc, in_maps, core_ids=list(range(NCORES)), trace=False
    )
    return np.concatenate([res.results[c]["out"] for c in range(NCORES)], axis=0)
